# revision 21
# baseline (speedup 1.0000x reference)
"""DualStreamStegoDiSBlock on 8 TRN2 NeuronCores (Bass/Tile).

Sharding: tensor-parallel over d_inner (DI=2048 -> 256 ch/core) for both
mambas; x_proj/out_proj produce partials that are AllReduced. Everything
else (pointwise matmuls, final gating) is computed redundantly on every
core; host reads core 0's output.

All DRAM matrices [R, C] are stored interleaved as [128, R//128, C] with
row r at (p=r%128, rr=r//128)  (matmul_tile_kernel's native layout).
"""

import sys

sys.path.insert(0, "/opt/trn_rl_repo")

import numpy as np

DIM = 1024
SECRET = 512
B = 4
L = 1024
DI = 2 * DIM
DSTATE = 16
DTRANK = DIM // 16
K = 4
EPS = 1e-5
NC = 8
DSH = DI // NC  # 256 d_inner channels per core
T = B * L  # 4096 tokens
P = 128


def _pm(a):
    """[R, C] -> [128, R//128, C] row-interleaved (r -> (r%128, r//128))."""
    r, c = a.shape
    assert r % P == 0, r
    return np.ascontiguousarray(a.reshape(r // P, P, c).transpose(1, 0, 2))


def _unpm(a):
    """inverse of _pm"""
    p, rr, c = a.shape
    return np.ascontiguousarray(a.transpose(1, 0, 2).reshape(rr * p, c))


def _bf16(a):
    import ml_dtypes

    return np.asarray(a, dtype=ml_dtypes.bfloat16)


def _silu_np(x):
    return x / (1.0 + np.exp(-x))


def _build_bass(alpha_x: float, alpha_z: float):
    import concourse.bass as bass
    import concourse.mybir as mybir
    import concourse.tile as tile
    from concourse import bacc
    from concourse.alu_op_type import AluOpType
    from concourse.kernels.tile_matmul import matmul_tile_kernel
    from concourse.masks import make_identity

    f32 = mybir.dt.float32
    bf16 = mybir.dt.bfloat16
    AF = mybir.ActivationFunctionType
    groups = [list(range(NC))]

    nc = bacc.Bacc(None, target_bir_lowering=False)

    with tile.TileContext(nc) as tc:
        dram = tc.alloc_tile_pool(name="dram", bufs=1, space="DRAM")

        def din(name, shape, dt=f32):
            return dram.tile(shape, dt, kind="ExternalInput", name=name,
                             uniquify=False)

        # ---- inputs (per-core content differs only for weight slices) ----
        smod = din("smod", [P, 8, T], bf16)      # sem_mod^T
        tmod = din("tmod", [P, 8, T], bf16)      # tex_mod^T
        sec = din("sec", [P, 4, T], bf16)        # secret^T
        hsem = din("hsem", [P, 8, T], bf16)      # h_sem^T
        htex = din("htex", [P, 8, T], bf16)      # h_tex^T
        w_in = {m: din(f"w_in_{m}", [P, 8, 2 * DSH], bf16) for m in "st"}
        cfgs = ["sf", "sb", "tf", "tb"]
        xpw = {c: din(f"xpw_{c}", [P, 2, P], bf16) for c in cfgs}
        dtw = {c: din(f"dtw_{c}", [P, 1, DSH], bf16) for c in cfgs}
        convw = {c: din(f"convw_{c}", [P, 2, K]) for c in cfgs}
        convb = {c: din(f"convb_{c}", [P, 2, 1]) for c in cfgs}
        dtb = {c: din(f"dtb_{c}", [P, 2, 1]) for c in cfgs}
        A_in = {c: din(f"A_{c}", [P, 2, DSTATE]) for c in cfgs}
        D_in = {c: din(f"D_{c}", [P, 2, 1]) for c in cfgs}
        wout = {m: din(f"wout_{m}", [P, 2, DIM], bf16) for m in "st"}
        wsx = din("wsx", [P, 4, DIM], bf16)
        bsx = din("bsx", [P, 8, 1])
        wsz = din("wsz", [P, 4, DIM], bf16)
        bsz = din("bsz", [P, 8, 1])
        wtz = din("wtz", [P, 8, DIM], bf16)
        btz = din("btz", [P, 8, 1])
        wmask = din("wmask", [P, 8, DIM], bf16)
        bmask = din("bmask", [P, 8, 1])
        gs = din("gs", [P, 8, B])
        gt = din("gt", [P, 8, B])

        out_sem = dram.tile([P, 8, T], f32, kind="ExternalOutput",
                            name="out_sem", uniquify=False)
        out_tex = dram.tile([P, 8, T], f32, kind="ExternalOutput",
                            name="out_tex", uniquify=False)

        # ---- DRAM intermediates ----
        xT = dram.tile([P, 8, T], bf16, name="xT")
        xz = {"s": dram.tile([P, 4, T], f32, name="xz_s"),
              "t": dram.tile([P, 4, T], f32, name="xz_t")}
        u_d = {c: dram.tile([P, 2, T], f32, name=f"u_{c}") for c in cfgs}
        dblps = {c: dram.tile([P, 1, T], f32, name=f"dblp_{c}")
                 for c in cfgs}
        dbl_ars = {c: dram.tile([P, 1, T], f32, name=f"dblar_{c}",
                                addr_space="Shared") for c in cfgs}
        delta = {c: dram.tile([P, 2, T], f32, name=f"delta_{c}") for c in cfgs}
        y_d = {c: dram.tile([P, 2, T], f32, name=f"y_{c}") for c in cfgs}
        ygate = {m: dram.tile([P, 2, T], bf16, name=f"ygate_{m}") for m in "st"}
        Oalls = {m: dram.tile([P, 8, T], f32, name=f"Oall_{m}")
                 for m in "st"}
        Oars = {m: dram.tile([P, 8, T], f32, name=f"Oar_{m}",
                             addr_space="Shared") for m in "st"}
        maskin = dram.tile([P, 8, T], bf16, name="maskin")
        maskT = dram.tile([P, 8, T], bf16, name="maskT")
        z1 = dram.tile([P, 8, T], f32, name="z1")
        z2 = dram.tile([P, 8, T], f32, name="z2")

        const = tc.alloc_tile_pool(name="const", bufs=1)

        _cn = [0]

        def _load_const(ap, shape, dt=f32):
            _cn[0] += 1
            t_ = const.tile(shape, dt, tag=f"c{_cn[0]}")
            nc.sync.dma_start(out=t_[:], in_=ap)
            return t_

        ones1 = const.tile([P, P], f32)
        nc.any.memset(ones1[:], 1.0)
        onec = const.tile([P, 1], f32, tag="onec")
        nc.any.memset(onec[:], 1.0)
        ident = const.tile([P, P], bf16, tag="ident")
        make_identity(nc, ident)

        def post_act(bias_sb, func, scale_after=None):
            """post_mxn hook: sbuf[:, j, :] = func(sbuf[:, j, :] + bias[:, col])
            (bias_sb is an SBUF const tile [P, 8, 1])."""
            def fn(nc_, sbuf, md, _):
                msub = sbuf.shape[1] if len(sbuf.shape) == 3 else 1
                for j in range(msub):
                    col = (md.m_tile_idx * md.m_tile) // P + j
                    sl = sbuf[:, j, :] if len(sbuf.shape) == 3 else sbuf[:]
                    if func == "softplus":  # ln(1 + exp(x + b))
                        nc_.scalar.activation(sl, sl, AF.Exp,
                                              bias=bias_sb[:, col, :])
                        nc_.scalar.activation(sl, sl, AF.Ln, bias=onec[:])
                    else:
                        nc_.scalar.activation(sl, sl, func,
                                              bias=bias_sb[:, col, :])
                    if scale_after is not None:
                        nc_.scalar.mul(sl, sl, scale_after)
            return fn

        bsx_sb = _load_const(bsx[:], [P, 8, 1])
        bsz_sb = _load_const(bsz[:], [P, 8, 1])
        btz_sb = _load_const(btz[:], [P, 8, 1])
        bmask_sb = _load_const(bmask[:], [P, 8, 1])
        dtb_sb = {c: _load_const(dtb[c][:], [P, 2, 1]) for c in cfgs}
        convw_sb = {c: _load_const(convw[c][:], [P, 2, K]) for c in cfgs}
        convb_sb = {c: _load_const(convb[c][:], [P, 2, 1]) for c in cfgs}
        A_sb = {c: _load_const(A_in[c][:], [P, 2, DSTATE]) for c in cfgs}
        D_sb = {c: _load_const(D_in[c][:], [P, 2, 1]) for c in cfgs}
        gs_sb = _load_const(gs[:], [P, 8, B])
        gt_sb = _load_const(gt[:], [P, 8, B])

        # ---- phase 1: x = tmod + ax*tanh(Wsx @ sec + bsx) ----
        matmul_tile_kernel(tc, wsx[:], sec[:], xT[:], accumulate_ap=tmod[:],
                           post_mxn_tile_fn=post_act(bsx_sb, AF.Tanh, alpha_x))
        # in_proj (sem from smod, tex from x)
        matmul_tile_kernel(tc, w_in["s"][:], smod[:], xz["s"][:])
        matmul_tile_kernel(tc, w_in["t"][:], xT[:], xz["t"][:])

        # ---- phase 2: causal depthwise conv + silu (bwd cfgs reversed) ----
        cpool = tc.alloc_tile_pool(name="conv", bufs=3)
        for c in cfgs:
            src = xz[c[0] == "t" and "t" or "s"]
            rev = c[1] == "b"
            for kk in range(2):
                for b in range(B):
                    sl = slice(b * L, (b + 1) * L)
                    X = cpool.tile([P, L + K - 1], f32, tag="X")
                    nc.any.memset(X[:, 0:K - 1], 0.0)
                    src_ap = src[:, kk, sl]
                    if rev:
                        src_ap = src_ap[:, ::-1]
                    nc.sync.dma_start(out=X[:, K - 1:], in_=src_ap)
                    acc = cpool.tile([P, L], f32, tag="acc")
                    nc.vector.tensor_scalar_mul(
                        acc[:], X[:, 0:L], convw_sb[c][:, kk, 0:1])
                    for k in range(1, K):
                        nc.vector.scalar_tensor_tensor(
                            acc[:], X[:, k:k + L], convw_sb[c][:, kk, k:k + 1],
                            acc[:], AluOpType.mult, AluOpType.add)
                    us = cpool.tile([P, L], f32, tag="us")
                    nc.scalar.activation(us[:], acc[:], AF.Silu,
                                         bias=convb_sb[c][:, kk, :])
                    nc.sync.dma_start(out=u_d[c][:, kk, sl], in_=us[:])

        # ---- phase 3: x_proj partials then AllReduce ----
        for i, c in enumerate(cfgs):
            matmul_tile_kernel(tc, xpw[c][:], u_d[c][:], dblps[c][:],
                               matmul_dtype=bf16)
            nc.gpsimd.collective_compute(
                "AllReduce", AluOpType.add, replica_groups=groups,
                ins=[dblps[c][:]], outs=[dbl_ars[c][:]])

        # ---- phase 4: delta = softplus(dtw @ dt + dtb) ----
        for i, c in enumerate(cfgs):
            matmul_tile_kernel(tc, dtw[c][:], dbl_ars[c][:],
                               delta[c][:], matmul_dtype=bf16,
                               post_mxn_tile_fn=post_act(dtb_sb[c],
                                                         "softplus"))

        # ---- phase 5: selective scan ----
        cpool.release()
        spool = tc.alloc_tile_pool(name="scan", bufs=3)
        sres = tc.alloc_tile_pool(name="scan_res", bufs=2)
        pp = tc.alloc_tile_pool(name="scan_psum", bufs=1, space="PSUM")
        ppy = tc.alloc_tile_pool(name="scan_psum_y", bufs=1, space="PSUM")
        for i, c in enumerate(cfgs):
            for b in range(B):
                sl = slice(b * L, (b + 1) * L)
                dl, du, ya = [], [], []
                for kk in range(2):
                    d_ = sres.tile([P, L], f32, tag=f"d{kk}")
                    u_ = sres.tile([P, L], f32, tag=f"u{kk}")
                    nc.sync.dma_start(out=d_[:], in_=delta[c][:, kk, sl])
                    nc.sync.dma_start(out=u_[:], in_=u_d[c][:, kk, sl])
                    duk = sres.tile([P, L], f32, tag=f"du{kk}")
                    nc.vector.tensor_mul(duk[:], d_[:], u_[:])
                    y_ = ppy.tile([P, L], f32, tag=f"yp{kk}")
                    dl.append(d_); du.append(duk); ya.append(y_)
                for n in range(DSTATE):
                    Bn = spool.tile([1, 2, L], f32, tag="Bn")  # B_n, C_n rows
                    nc.sync.dma_start(
                        out=Bn[:],
                        in_=dbl_ars[c][DTRANK + n:DTRANK + n + DSTATE + 1:
                                       DSTATE, 0, sl])
                    pb = pp.tile([P, L], f32, tag="pb")
                    pc = pp.tile([P, L], f32, tag="pc")
                    for h in range(2):
                        hs = slice(h * 512, (h + 1) * 512)
                        nc.tensor.matmul(pb[:, hs], ones1[0:1, :],
                                         Bn[0:1, 0, hs])
                        nc.tensor.matmul(pc[:, hs], ones1[0:1, :],
                                         Bn[0:1, 1, hs])
                    for kk in range(2):
                        a_ = spool.tile([P, L], f32, tag="a")
                        nc.scalar.activation(a_[:], dl[kk][:], AF.Exp,
                                             scale=A_sb[c][:, kk, n:n + 1])
                        bt = spool.tile([P, L], f32, tag="bt")
                        nc.vector.tensor_mul(bt[:], du[kk][:], pb[:])
                        h_ = spool.tile([P, L], f32, tag="h")
                        nc.vector.tensor_tensor_scan(
                            h_[:], a_[:], bt[:], 0.0,
                            AluOpType.mult, AluOpType.add)
                        yc = spool.tile([P, L], bf16, tag="yc")
                        nc.vector.tensor_mul(yc[:], h_[:], pc[:])
                        first = n == 0
                        last = n == DSTATE - 1
                        for hh in range(2):
                            hs2 = slice(hh * 512, (hh + 1) * 512)
                            nc.tensor.matmul(ya[kk][:, hs2], ident[:],
                                             yc[:, hs2], start=first,
                                             stop=last)
                for kk in range(2):
                    yo = spool.tile([P, L], f32, tag="yo")
                    nc.scalar.copy(yo[:], ya[kk][:])
                    nc.sync.dma_start(out=y_d[c][:, kk, sl], in_=yo[:])

        # ---- phase 6: y_total = (yf + uf*Df + rev(yb + ub*Db)) * silu(z) ----
        ppy.release(); pp.release(); sres.release(); spool.release()
        gpool = tc.alloc_tile_pool(name="gate", bufs=3)
        for m in "st":
            cf, cb = m + "f", m + "b"
            for kk in range(2):
                for b in range(B):
                    sl = slice(b * L, (b + 1) * L)
                    yf = gpool.tile([P, L], f32, tag="yf")
                    uf = gpool.tile([P, L], f32, tag="uf")
                    yb = gpool.tile([P, L], f32, tag="yb")
                    ub = gpool.tile([P, L], f32, tag="ub")
                    nc.sync.dma_start(out=yf[:], in_=y_d[cf][:, kk, sl])
                    nc.sync.dma_start(out=uf[:], in_=u_d[cf][:, kk, sl])
                    nc.sync.dma_start(out=yb[:], in_=y_d[cb][:, kk, sl][:, ::-1])
                    nc.sync.dma_start(out=ub[:], in_=u_d[cb][:, kk, sl][:, ::-1])
                    nc.vector.scalar_tensor_tensor(
                        yf[:], uf[:], D_sb[cf][:, kk, :], yf[:],
                        AluOpType.mult, AluOpType.add)
                    nc.vector.scalar_tensor_tensor(
                        yb[:], ub[:], D_sb[cb][:, kk, :], yb[:],
                        AluOpType.mult, AluOpType.add)
                    nc.vector.tensor_add(yf[:], yf[:], yb[:])
                    zz = gpool.tile([P, L], f32, tag="zz")
                    nc.sync.dma_start(out=zz[:],
                                      in_=xz[m][:, 2 + kk, sl])
                    nc.scalar.activation(zz[:], zz[:], AF.Silu)
                    yg = gpool.tile([P, L], bf16, tag="yg")
                    nc.vector.tensor_mul(yg[:], yf[:], zz[:])
                    nc.sync.dma_start(out=ygate[m][:, kk, sl], in_=yg[:])

        # ---- phase 7: out_proj partials + AllReduce ----
        matmul_tile_kernel(tc, wout["s"][:], ygate["s"][:], Oalls["s"][:])
        nc.gpsimd.collective_compute(
            "AllReduce", AluOpType.add, replica_groups=groups,
            ins=[Oalls["s"][:]], outs=[Oars["s"][:]])
        matmul_tile_kernel(tc, wout["t"][:], ygate["t"][:], Oalls["t"][:])
        nc.gpsimd.collective_compute(
            "AllReduce", AluOpType.add, replica_groups=groups,
            ins=[Oalls["t"][:]], outs=[Oars["t"][:]])

        # ---- phase 8: h_sem_out (= mask input), mask, z, final gating ----
        gpool.release()
        fpool = tc.alloc_tile_pool(name="fin", bufs=3)
        for rr in range(8):
            for b in range(B):
                sl = slice(b * L, (b + 1) * L)
                o_ = fpool.tile([P, L], f32, tag="o")
                hs = fpool.tile([P, L], bf16, tag="hs")
                nc.sync.dma_start(out=o_[:], in_=Oars["s"][:, rr, sl])
                nc.sync.dma_start(out=hs[:], in_=hsem[:, rr, sl])
                ho = fpool.tile([P, L], f32, tag="ho")
                nc.vector.scalar_tensor_tensor(
                    ho[:], o_[:], gs_sb[:, rr, b:b + 1], hs[:],
                    AluOpType.mult, AluOpType.add)
                nc.sync.dma_start(out=out_sem[:, rr, sl], in_=ho[:])
                hob = fpool.tile([P, L], bf16, tag="hob")
                nc.vector.tensor_copy(hob[:], ho[:])
                nc.sync.dma_start(out=maskin[:, rr, sl], in_=hob[:])

        matmul_tile_kernel(tc, wmask[:], maskin[:], maskT[:],
                           post_mxn_tile_fn=post_act(bmask_sb, AF.Sigmoid))
        matmul_tile_kernel(tc, wtz[:], tmod[:], z1[:],
                           post_mxn_tile_fn=post_act(btz_sb, AF.Identity))
        matmul_tile_kernel(tc, wsz[:], sec[:], z2[:],
                           post_mxn_tile_fn=post_act(bsz_sb, AF.Tanh, alpha_z))

        for rr in range(8):
            for b in range(B):
                sl = slice(b * L, (b + 1) * L)
                za = fpool.tile([P, L], f32, tag="za")
                zb = fpool.tile([P, L], f32, tag="zb")
                nc.sync.dma_start(out=za[:], in_=z1[:, rr, sl])
                nc.sync.dma_start(out=zb[:], in_=z2[:, rr, sl])
                nc.vector.tensor_add(za[:], za[:], zb[:])
                nc.scalar.activation(za[:], za[:], AF.Silu)
                ot = fpool.tile([P, L], f32, tag="ot")
                nc.sync.dma_start(out=ot[:], in_=Oars["t"][:, rr, sl])
                nc.vector.tensor_mul(ot[:], ot[:], za[:])
                mk = fpool.tile([P, L], bf16, tag="mk")
                nc.sync.dma_start(out=mk[:], in_=maskT[:, rr, sl])
                nc.vector.tensor_mul(ot[:], ot[:], mk[:])
                ht = fpool.tile([P, L], bf16, tag="ht")
                nc.sync.dma_start(out=ht[:], in_=htex[:, rr, sl])
                hto = fpool.tile([P, L], f32, tag="hto")
                nc.vector.scalar_tensor_tensor(
                    hto[:], ot[:], gt_sb[:, rr, b:b + 1], ht[:],
                    AluOpType.mult, AluOpType.add)
                nc.sync.dma_start(out=out_tex[:, rr, sl], in_=hto[:])

        fpool.release()
        const.release()

    nc.compile()
    return nc


_CACHE = {}


def kernel(h_sem, h_tex, c_global, secret_seq, params, sem_mamba, tex_mamba):
    p = params
    # ---- host: adaLN + LN + modulation (tiny fraction of FLOPs) ----
    c = _silu_np(np.asarray(c_global, np.float32))
    mod_s = c @ np.asarray(p["adaLN_sem_w"], np.float32).T + np.asarray(
        p["adaLN_sem_b"], np.float32)
    mod_t = c @ np.asarray(p["adaLN_tex_w"], np.float32).T + np.asarray(
        p["adaLN_tex_b"], np.float32)
    shift_s, scale_s, gate_s = np.split(mod_s, 3, -1)
    shift_t, scale_t, gate_t = np.split(mod_t, 3, -1)

    def ln_mod(h, w, b, scale, shift):
        h = np.asarray(h, np.float32)
        mu = h.mean(-1, keepdims=True)
        var = h.var(-1, keepdims=True)
        xh = (h - mu) / np.sqrt(var + EPS) * np.asarray(w, np.float32) + \
            np.asarray(b, np.float32)
        return xh * (1.0 + scale[:, None]) + shift[:, None]

    sem_mod = ln_mod(h_sem, p["norm_sem_w"], p["norm_sem_b"], scale_s, shift_s)
    tex_mod = ln_mod(h_tex, p["norm_tex_w"], p["norm_tex_b"], scale_t, shift_t)

    def tokmat(x):  # (B,L,D) -> D-major [P, D//P, T]
        d = x.shape[-1]
        return _pm(np.asarray(x, np.float32).reshape(T, d).T)

    smod_T = _bf16(tokmat(sem_mod))
    tmod_T = _bf16(tokmat(tex_mod))
    sec_T = _bf16(tokmat(secret_seq))
    hsem_T = _bf16(tokmat(h_sem))
    htex_T = _bf16(tokmat(h_tex))

    base = {
        "smod": smod_T, "tmod": tmod_T, "sec": sec_T,
        "hsem": hsem_T, "htex": htex_T,
        "wsx": _bf16(_pm(np.asarray(p["secret_to_x_w"], np.float32).T)),
        "bsx": _pm(np.asarray(p["secret_to_x_b"], np.float32)[:, None]),
        "wsz": _bf16(_pm(np.asarray(p["secret_to_z_w"], np.float32).T)),
        "bsz": _pm(np.asarray(p["secret_to_z_b"], np.float32)[:, None]),
        "wtz": _bf16(_pm(np.asarray(p["tex_to_z_w"], np.float32).T)),
        "btz": _pm(np.asarray(p["tex_to_z_b"], np.float32)[:, None]),
        "wmask": _bf16(_pm(np.asarray(p["mask_proj_w"], np.float32).T)),
        "bmask": _pm(np.asarray(p["mask_proj_b"], np.float32)[:, None]),
        "gs": _pm(np.ascontiguousarray(gate_s.T)),
        "gt": _pm(np.ascontiguousarray(gate_t.T)),
    }

    in_maps = []
    for ci in range(NC):
        dsl = slice(ci * DSH, (ci + 1) * DSH)
        m = dict(base)
        for mk_, mp in (("s", sem_mamba), ("t", tex_mamba)):
            wi = np.asarray(mp["in_proj_w"], np.float32)
            m[f"w_in_{mk_}"] = _bf16(_pm(np.ascontiguousarray(
                np.concatenate([wi[dsl], wi[DI + ci * DSH: DI + (ci + 1) * DSH]],
                               0).T)))
            wo = np.asarray(mp["out_proj_w"], np.float32)
            m[f"wout_{mk_}"] = _bf16(_pm(np.ascontiguousarray(wo[:, dsl].T)))
            for br, s in (("f", ""), ("b", "_b")):
                cname = mk_ + br
                xw = np.asarray(mp["xproj_w" + s], np.float32)[:, dsl]  # 96x256
                xw_pad = np.zeros((P, DSH), np.float32)
                xw_pad[:96] = xw
                m[f"xpw_{cname}"] = _bf16(_pm(np.ascontiguousarray(xw_pad.T)))
                dw = np.asarray(mp["dt_w" + s], np.float32)[dsl]  # 256 x 64
                dw_pad = np.zeros((P, DSH), np.float32)
                dw_pad[:DTRANK] = dw.T
                m[f"dtw_{cname}"] = _bf16(dw_pad[:, None, :])
                cw = np.asarray(mp["conv_w" + s], np.float32)[dsl]  # 256x4
                m[f"convw_{cname}"] = _pm(cw).astype(np.float32)
                m[f"convb_{cname}"] = _pm(
                    np.asarray(mp["conv_b" + s], np.float32)[dsl, None])
                m[f"dtb_{cname}"] = _pm(
                    np.asarray(mp["dt_bias" + s], np.float32)[dsl, None])
                m[f"A_{cname}"] = _pm(-np.exp(
                    np.asarray(mp["A_log" + s], np.float32)[dsl]))
                m[f"D_{cname}"] = _pm(
                    np.asarray(mp["D" + s], np.float32)[dsl, None])
        in_maps.append(m)

    key = "nc"
    if key not in _CACHE:
        _CACHE[key] = _build_bass(float(np.asarray(params["alpha_x"])),
                                  float(np.asarray(params["alpha_z"])))
    nc = _CACHE[key]

    from concourse.bass_utils import run_bass_kernel_spmd
    import os as _os
    res = run_bass_kernel_spmd(nc, in_maps, core_ids=list(range(NC)),
                               trace=bool(_os.environ.get("BASS_TRACE")))
    if res.exec_time_ns is not None:
        print(f"HW exec time: {res.exec_time_ns} ns")
    else:
        try:  # no NTFF hook under this axon client: report cost-model estimate
            from concourse.timeline_sim import TimelineSim
            est = TimelineSim(nc, trace=False).simulate()
            print(f"HW exec time: {est:.0f} ns (TimelineSim estimate)")
        except Exception:
            pass
    r0 = res.results[0]
    hs_out = _unpm(np.asarray(r0["out_sem"], np.float32)).T.reshape(B, L, DIM)
    ht_out = _unpm(np.asarray(r0["out_tex"], np.float32)).T.reshape(B, L, DIM)
    return hs_out, ht_out


# revision 27
# speedup vs baseline: 1.0306x; 1.0306x over previous
"""DualStreamStegoDiSBlock on 8 TRN2 NeuronCores (Bass/Tile).

Sharding: tensor-parallel over d_inner (DI=2048 -> 256 ch/core) for both
mambas; x_proj/out_proj produce partials that are AllReduced. Everything
else (pointwise matmuls, final gating) is computed redundantly on every
core; host reads core 0's output.

All DRAM matrices [R, C] are stored interleaved as [128, R//128, C] with
row r at (p=r%128, rr=r//128)  (matmul_tile_kernel's native layout).
"""

import sys

sys.path.insert(0, "/opt/trn_rl_repo")

import numpy as np

DIM = 1024
SECRET = 512
B = 4
L = 1024
DI = 2 * DIM
DSTATE = 16
DTRANK = DIM // 16
K = 4
EPS = 1e-5
NC = 8
DSH = DI // NC  # 256 d_inner channels per core
T = B * L  # 4096 tokens
P = 128


def _pm(a):
    """[R, C] -> [128, R//128, C] row-interleaved (r -> (r%128, r//128))."""
    r, c = a.shape
    assert r % P == 0, r
    return np.ascontiguousarray(a.reshape(r // P, P, c).transpose(1, 0, 2))


def _unpm(a):
    """inverse of _pm"""
    p, rr, c = a.shape
    return np.ascontiguousarray(a.transpose(1, 0, 2).reshape(rr * p, c))


def _bf16(a):
    import ml_dtypes

    return np.asarray(a, dtype=ml_dtypes.bfloat16)


def _silu_np(x):
    return x / (1.0 + np.exp(-x))


def _build_bass(alpha_x: float, alpha_z: float):
    import concourse.bass as bass
    import concourse.mybir as mybir
    import concourse.tile as tile
    from concourse import bacc
    from concourse.alu_op_type import AluOpType
    from concourse.kernels.tile_matmul import matmul_tile_kernel
    from concourse.masks import make_identity

    f32 = mybir.dt.float32
    bf16 = mybir.dt.bfloat16
    AF = mybir.ActivationFunctionType
    groups = [list(range(NC))]

    nc = bacc.Bacc(None, target_bir_lowering=False)

    with tile.TileContext(nc) as tc:
        dram = tc.alloc_tile_pool(name="dram", bufs=1, space="DRAM")

        def din(name, shape, dt=f32):
            return dram.tile(shape, dt, kind="ExternalInput", name=name,
                             uniquify=False)

        # ---- inputs (per-core content differs only for weight slices) ----
        smod = din("smod", [P, 8, T], bf16)      # sem_mod^T
        tmod = din("tmod", [P, 8, T], bf16)      # tex_mod^T
        sec = din("sec", [P, 4, T], bf16)        # secret^T
        hsem = din("hsem", [P, 8, T], bf16)      # h_sem^T
        htex = din("htex", [P, 8, T], bf16)      # h_tex^T
        w_in = {m: din(f"w_in_{m}", [P, 8, 2 * DSH], bf16) for m in "st"}
        cfgs = ["sf", "sb", "tf", "tb"]
        xpw = {c: din(f"xpw_{c}", [P, 2, P], bf16) for c in cfgs}
        dtw = {c: din(f"dtw_{c}", [P, 1, DSH], bf16) for c in cfgs}
        convw = {c: din(f"convw_{c}", [P, 2, K]) for c in cfgs}
        convb = {c: din(f"convb_{c}", [P, 2, 1]) for c in cfgs}
        dtb = {c: din(f"dtb_{c}", [P, 2, 1]) for c in cfgs}
        A_in = {c: din(f"A_{c}", [P, 2, DSTATE]) for c in cfgs}
        D_in = {c: din(f"D_{c}", [P, 2, 1]) for c in cfgs}
        wout = {m: din(f"wout_{m}", [P, 2, DIM], bf16) for m in "st"}
        wsx = din("wsx", [P, 4, DIM], bf16)
        bsx = din("bsx", [P, 8, 1])
        wsz = din("wsz", [P, 4, DIM], bf16)
        bsz = din("bsz", [P, 8, 1])
        wtz = din("wtz", [P, 8, DIM], bf16)
        btz = din("btz", [P, 8, 1])
        wmask = din("wmask", [P, 8, DIM], bf16)
        bmask = din("bmask", [P, 8, 1])
        gs = din("gs", [P, 8, B])
        gt = din("gt", [P, 8, B])

        out_sem = dram.tile([P, 8, T], f32, kind="ExternalOutput",
                            name="out_sem", uniquify=False)
        out_tex = dram.tile([P, 8, T], f32, kind="ExternalOutput",
                            name="out_tex", uniquify=False)

        # ---- DRAM intermediates ----
        xT = dram.tile([P, 8, T], bf16, name="xT")
        xz = {"s": dram.tile([P, 4, T], f32, name="xz_s"),
              "t": dram.tile([P, 4, T], f32, name="xz_t")}
        u_d = {c: dram.tile([P, 2, T], f32, name=f"u_{c}") for c in cfgs}
        dblps = {c: dram.tile([P, 1, T], f32, name=f"dblp_{c}")
                 for c in cfgs}
        dbl_ars = {c: dram.tile([P, 1, T], f32, name=f"dblar_{c}",
                                addr_space="Shared") for c in cfgs}
        delta = {c: dram.tile([P, 2, T], f32, name=f"delta_{c}") for c in cfgs}
        y_d = {c: dram.tile([P, 2, T], f32, name=f"y_{c}") for c in cfgs}
        ygate = {m: dram.tile([P, 2, T], bf16, name=f"ygate_{m}") for m in "st"}
        Oalls = {m: dram.tile([P, 8, T], bf16, name=f"Oall_{m}")
                 for m in "st"}
        Oars = {m: dram.tile([P, 8, T], bf16, name=f"Oar_{m}",
                             addr_space="Shared") for m in "st"}
        maskin = dram.tile([P, 8, T], bf16, name="maskin")
        maskT = dram.tile([P, 8, T], bf16, name="maskT")
        z1 = dram.tile([P, 8, T], f32, name="z1")
        z2 = dram.tile([P, 8, T], f32, name="z2")

        const = tc.alloc_tile_pool(name="const", bufs=1)

        _cn = [0]

        def _load_const(ap, shape, dt=f32):
            _cn[0] += 1
            t_ = const.tile(shape, dt, tag=f"c{_cn[0]}")
            nc.sync.dma_start(out=t_[:], in_=ap)
            return t_

        ones1 = const.tile([P, P], bf16)
        nc.any.memset(ones1[:], 1.0)
        onec = const.tile([P, 1], f32, tag="onec")
        nc.any.memset(onec[:], 1.0)
        ident = const.tile([P, P], bf16, tag="ident")
        make_identity(nc, ident)

        def post_act(bias_sb, func, scale_after=None):
            """post_mxn hook: sbuf[:, j, :] = func(sbuf[:, j, :] + bias[:, col])
            (bias_sb is an SBUF const tile [P, 8, 1])."""
            def fn(nc_, sbuf, md, _):
                msub = sbuf.shape[1] if len(sbuf.shape) == 3 else 1
                for j in range(msub):
                    col = (md.m_tile_idx * md.m_tile) // P + j
                    sl = sbuf[:, j, :] if len(sbuf.shape) == 3 else sbuf[:]
                    if func == "softplus":  # ln(1 + exp(x + b))
                        nc_.scalar.activation(sl, sl, AF.Exp,
                                              bias=bias_sb[:, col, :])
                        nc_.scalar.activation(sl, sl, AF.Ln, bias=onec[:])
                    else:
                        nc_.scalar.activation(sl, sl, func,
                                              bias=bias_sb[:, col, :])
                    if scale_after is not None:
                        nc_.scalar.mul(sl, sl, scale_after)
            return fn

        bsx_sb = _load_const(bsx[:], [P, 8, 1])
        bsz_sb = _load_const(bsz[:], [P, 8, 1])
        btz_sb = _load_const(btz[:], [P, 8, 1])
        bmask_sb = _load_const(bmask[:], [P, 8, 1])
        dtb_sb = {c: _load_const(dtb[c][:], [P, 2, 1]) for c in cfgs}
        convw_sb = {c: _load_const(convw[c][:], [P, 2, K]) for c in cfgs}
        convb_sb = {c: _load_const(convb[c][:], [P, 2, 1]) for c in cfgs}
        A_sb = {c: _load_const(A_in[c][:], [P, 2, DSTATE]) for c in cfgs}
        D_sb = {c: _load_const(D_in[c][:], [P, 2, 1]) for c in cfgs}
        gs_sb = _load_const(gs[:], [P, 8, B])
        gt_sb = _load_const(gt[:], [P, 8, B])

        # ---- phase 1: x = tmod + ax*tanh(Wsx @ sec + bsx) ----
        matmul_tile_kernel(tc, wsx[:], sec[:], xT[:], accumulate_ap=tmod[:],
                           post_mxn_tile_fn=post_act(bsx_sb, AF.Tanh, alpha_x))
        # in_proj (sem from smod, tex from x)
        matmul_tile_kernel(tc, w_in["s"][:], smod[:], xz["s"][:])
        matmul_tile_kernel(tc, w_in["t"][:], xT[:], xz["t"][:])

        # ---- phase 2: causal depthwise conv + silu (bwd cfgs reversed) ----
        cpool = tc.alloc_tile_pool(name="conv", bufs=3)
        for c in cfgs:
            src = xz[c[0] == "t" and "t" or "s"]
            rev = c[1] == "b"
            for kk in range(2):
                for b in range(B):
                    sl = slice(b * L, (b + 1) * L)
                    X = cpool.tile([P, L + K - 1], f32, tag="X")
                    nc.any.memset(X[:, 0:K - 1], 0.0)
                    src_ap = src[:, kk, sl]
                    if rev:
                        src_ap = src_ap[:, ::-1]
                    nc.sync.dma_start(out=X[:, K - 1:], in_=src_ap)
                    acc = cpool.tile([P, L], f32, tag="acc")
                    nc.vector.tensor_scalar_mul(
                        acc[:], X[:, 0:L], convw_sb[c][:, kk, 0:1])
                    for k in range(1, K):
                        nc.vector.scalar_tensor_tensor(
                            acc[:], X[:, k:k + L], convw_sb[c][:, kk, k:k + 1],
                            acc[:], AluOpType.mult, AluOpType.add)
                    us = cpool.tile([P, L], f32, tag="us")
                    nc.scalar.activation(us[:], acc[:], AF.Silu,
                                         bias=convb_sb[c][:, kk, :])
                    nc.sync.dma_start(out=u_d[c][:, kk, sl], in_=us[:])

        # ---- phase 3: x_proj partials then AllReduce ----
        for i, c in enumerate(cfgs):
            matmul_tile_kernel(tc, xpw[c][:], u_d[c][:], dblps[c][:],
                               matmul_dtype=bf16)
            nc.gpsimd.collective_compute(
                "AllReduce", AluOpType.add, replica_groups=groups,
                ins=[dblps[c][:]], outs=[dbl_ars[c][:]])

        # ---- phase 4: delta = softplus(dtw @ dt + dtb) ----
        for i, c in enumerate(cfgs):
            matmul_tile_kernel(tc, dtw[c][:], dbl_ars[c][:],
                               delta[c][:], matmul_dtype=bf16,
                               post_mxn_tile_fn=post_act(dtb_sb[c],
                                                         "softplus"))

        # ---- phase 5: selective scan ----
        cpool.release()
        spool = tc.alloc_tile_pool(name="scan", bufs=3)
        sres = tc.alloc_tile_pool(name="scan_res", bufs=2)
        pp = tc.alloc_tile_pool(name="scan_psum", bufs=2, space="PSUM")
        ppy = tc.alloc_tile_pool(name="scan_psum_y", bufs=1, space="PSUM")
        for i, c in enumerate(cfgs):
            for b in range(B):
                sl = slice(b * L, (b + 1) * L)
                Bc = spool.tile([1, 2 * DSTATE, L], bf16, tag="Bc", bufs=1)
                nc.gpsimd.dma_start(out=Bc[:],
                                    in_=dbl_ars[c][DTRANK:DTRANK + 2 * DSTATE,
                                                   0, sl])
                dl, du, ya = [], [], []
                for kk in range(2):
                    d_ = sres.tile([P, L], f32, tag=f"d{kk}")
                    u_ = sres.tile([P, L], f32, tag=f"u{kk}")
                    nc.sync.dma_start(out=d_[:], in_=delta[c][:, kk, sl])
                    nc.sync.dma_start(out=u_[:], in_=u_d[c][:, kk, sl])
                    duk = sres.tile([P, L], f32, tag=f"du{kk}")
                    nc.vector.tensor_mul(duk[:], d_[:], u_[:])
                    y_ = ppy.tile([P, L], f32, tag=f"yp{kk}")
                    dl.append(d_); du.append(duk); ya.append(y_)
                for n in range(DSTATE):
                    pb = pp.tile([P, L], f32, tag="pb", bufs=1)
                    pc = pp.tile([P, L], f32, tag="pc", bufs=1)
                    for h in range(2):
                        hs = slice(h * 512, (h + 1) * 512)
                        nc.tensor.matmul(pb[:, hs], ones1[0:1, :],
                                         Bc[0:1, n, hs])
                        nc.tensor.matmul(pc[:, hs], ones1[0:1, :],
                                         Bc[0:1, DSTATE + n, hs])
                    for kk in range(2):
                        a_ = spool.tile([P, L], f32, tag="a")
                        nc.scalar.activation(a_[:], dl[kk][:], AF.Exp,
                                             scale=A_sb[c][:, kk, n:n + 1])
                        bt = spool.tile([P, L], f32, tag="bt")
                        nc.vector.tensor_mul(bt[:], du[kk][:], pb[:])
                        h_ = spool.tile([P, L], f32, tag="h")
                        nc.vector.tensor_tensor_scan(
                            h_[:], a_[:], bt[:], 0.0,
                            AluOpType.mult, AluOpType.add)
                        yc = spool.tile([P, L], bf16, tag="yc")
                        nc.vector.tensor_mul(yc[:], h_[:], pc[:])
                        first = n == 0
                        last = n == DSTATE - 1
                        for hh in range(2):
                            hs2 = slice(hh * 512, (hh + 1) * 512)
                            nc.tensor.matmul(ya[kk][:, hs2], ident[:],
                                             yc[:, hs2], start=first,
                                             stop=last)
                for kk in range(2):
                    yo = spool.tile([P, L], f32, tag="yo")
                    nc.scalar.copy(yo[:], ya[kk][:])
                    nc.sync.dma_start(out=y_d[c][:, kk, sl], in_=yo[:])

        # ---- phase 6: y_total = (yf + uf*Df + rev(yb + ub*Db)) * silu(z) ----
        ppy.release(); pp.release(); sres.release(); spool.release()
        gpool = tc.alloc_tile_pool(name="gate", bufs=3)
        for m in "st":
            cf, cb = m + "f", m + "b"
            for kk in range(2):
                for b in range(B):
                    sl = slice(b * L, (b + 1) * L)
                    yf = gpool.tile([P, L], f32, tag="yf")
                    uf = gpool.tile([P, L], f32, tag="uf")
                    yb = gpool.tile([P, L], f32, tag="yb")
                    ub = gpool.tile([P, L], f32, tag="ub")
                    nc.sync.dma_start(out=yf[:], in_=y_d[cf][:, kk, sl])
                    nc.sync.dma_start(out=uf[:], in_=u_d[cf][:, kk, sl])
                    nc.sync.dma_start(out=yb[:], in_=y_d[cb][:, kk, sl][:, ::-1])
                    nc.sync.dma_start(out=ub[:], in_=u_d[cb][:, kk, sl][:, ::-1])
                    nc.vector.scalar_tensor_tensor(
                        yf[:], uf[:], D_sb[cf][:, kk, :], yf[:],
                        AluOpType.mult, AluOpType.add)
                    nc.vector.scalar_tensor_tensor(
                        yb[:], ub[:], D_sb[cb][:, kk, :], yb[:],
                        AluOpType.mult, AluOpType.add)
                    nc.vector.tensor_add(yf[:], yf[:], yb[:])
                    zz = gpool.tile([P, L], f32, tag="zz")
                    nc.sync.dma_start(out=zz[:],
                                      in_=xz[m][:, 2 + kk, sl])
                    nc.scalar.activation(zz[:], zz[:], AF.Silu)
                    yg = gpool.tile([P, L], bf16, tag="yg")
                    nc.vector.tensor_mul(yg[:], yf[:], zz[:])
                    nc.sync.dma_start(out=ygate[m][:, kk, sl], in_=yg[:])

        # ---- phase 7: out_proj partials + AllReduce ----
        matmul_tile_kernel(tc, wout["s"][:], ygate["s"][:], Oalls["s"][:])
        nc.gpsimd.collective_compute(
            "AllReduce", AluOpType.add, replica_groups=groups,
            ins=[Oalls["s"][:]], outs=[Oars["s"][:]])
        matmul_tile_kernel(tc, wout["t"][:], ygate["t"][:], Oalls["t"][:])
        nc.gpsimd.collective_compute(
            "AllReduce", AluOpType.add, replica_groups=groups,
            ins=[Oalls["t"][:]], outs=[Oars["t"][:]])

        # ---- phase 8: h_sem_out (= mask input), mask, z, final gating ----
        gpool.release()
        fpool = tc.alloc_tile_pool(name="fin", bufs=3)
        for rr in range(8):
            for b in range(B):
                sl = slice(b * L, (b + 1) * L)
                o_ = fpool.tile([P, L], bf16, tag="o")
                hs = fpool.tile([P, L], bf16, tag="hs")
                nc.sync.dma_start(out=o_[:], in_=Oars["s"][:, rr, sl])
                nc.sync.dma_start(out=hs[:], in_=hsem[:, rr, sl])
                ho = fpool.tile([P, L], f32, tag="ho")
                nc.vector.scalar_tensor_tensor(
                    ho[:], o_[:], gs_sb[:, rr, b:b + 1], hs[:],
                    AluOpType.mult, AluOpType.add)
                nc.sync.dma_start(out=out_sem[:, rr, sl], in_=ho[:])
                hob = fpool.tile([P, L], bf16, tag="hob")
                nc.vector.tensor_copy(hob[:], ho[:])
                nc.sync.dma_start(out=maskin[:, rr, sl], in_=hob[:])

        matmul_tile_kernel(tc, wmask[:], maskin[:], maskT[:],
                           post_mxn_tile_fn=post_act(bmask_sb, AF.Sigmoid))
        matmul_tile_kernel(tc, wtz[:], tmod[:], z1[:],
                           post_mxn_tile_fn=post_act(btz_sb, AF.Identity))
        matmul_tile_kernel(tc, wsz[:], sec[:], z2[:],
                           post_mxn_tile_fn=post_act(bsz_sb, AF.Tanh, alpha_z))

        for rr in range(8):
            for b in range(B):
                sl = slice(b * L, (b + 1) * L)
                za = fpool.tile([P, L], f32, tag="za")
                zb = fpool.tile([P, L], f32, tag="zb")
                nc.sync.dma_start(out=za[:], in_=z1[:, rr, sl])
                nc.sync.dma_start(out=zb[:], in_=z2[:, rr, sl])
                nc.vector.tensor_add(za[:], za[:], zb[:])
                nc.scalar.activation(za[:], za[:], AF.Silu)
                ot = fpool.tile([P, L], bf16, tag="ot")
                nc.sync.dma_start(out=ot[:], in_=Oars["t"][:, rr, sl])
                nc.vector.tensor_mul(ot[:], ot[:], za[:])
                mk = fpool.tile([P, L], bf16, tag="mk")
                nc.sync.dma_start(out=mk[:], in_=maskT[:, rr, sl])
                nc.vector.tensor_mul(ot[:], ot[:], mk[:])
                ht = fpool.tile([P, L], bf16, tag="ht")
                nc.sync.dma_start(out=ht[:], in_=htex[:, rr, sl])
                hto = fpool.tile([P, L], f32, tag="hto")
                nc.vector.scalar_tensor_tensor(
                    hto[:], ot[:], gt_sb[:, rr, b:b + 1], ht[:],
                    AluOpType.mult, AluOpType.add)
                nc.sync.dma_start(out=out_tex[:, rr, sl], in_=hto[:])

        fpool.release()
        const.release()

    nc.compile()
    return nc


_CACHE = {}


def kernel(h_sem, h_tex, c_global, secret_seq, params, sem_mamba, tex_mamba):
    p = params
    # ---- host: adaLN + LN + modulation (tiny fraction of FLOPs) ----
    c = _silu_np(np.asarray(c_global, np.float32))
    mod_s = c @ np.asarray(p["adaLN_sem_w"], np.float32).T + np.asarray(
        p["adaLN_sem_b"], np.float32)
    mod_t = c @ np.asarray(p["adaLN_tex_w"], np.float32).T + np.asarray(
        p["adaLN_tex_b"], np.float32)
    shift_s, scale_s, gate_s = np.split(mod_s, 3, -1)
    shift_t, scale_t, gate_t = np.split(mod_t, 3, -1)

    def ln_mod(h, w, b, scale, shift):
        h = np.asarray(h, np.float32)
        mu = h.mean(-1, keepdims=True)
        var = h.var(-1, keepdims=True)
        xh = (h - mu) / np.sqrt(var + EPS) * np.asarray(w, np.float32) + \
            np.asarray(b, np.float32)
        return xh * (1.0 + scale[:, None]) + shift[:, None]

    sem_mod = ln_mod(h_sem, p["norm_sem_w"], p["norm_sem_b"], scale_s, shift_s)
    tex_mod = ln_mod(h_tex, p["norm_tex_w"], p["norm_tex_b"], scale_t, shift_t)

    def tokmat(x):  # (B,L,D) -> D-major [P, D//P, T]
        d = x.shape[-1]
        return _pm(np.asarray(x, np.float32).reshape(T, d).T)

    smod_T = _bf16(tokmat(sem_mod))
    tmod_T = _bf16(tokmat(tex_mod))
    sec_T = _bf16(tokmat(secret_seq))
    hsem_T = _bf16(tokmat(h_sem))
    htex_T = _bf16(tokmat(h_tex))

    base = {
        "smod": smod_T, "tmod": tmod_T, "sec": sec_T,
        "hsem": hsem_T, "htex": htex_T,
        "wsx": _bf16(_pm(np.asarray(p["secret_to_x_w"], np.float32).T)),
        "bsx": _pm(np.asarray(p["secret_to_x_b"], np.float32)[:, None]),
        "wsz": _bf16(_pm(np.asarray(p["secret_to_z_w"], np.float32).T)),
        "bsz": _pm(np.asarray(p["secret_to_z_b"], np.float32)[:, None]),
        "wtz": _bf16(_pm(np.asarray(p["tex_to_z_w"], np.float32).T)),
        "btz": _pm(np.asarray(p["tex_to_z_b"], np.float32)[:, None]),
        "wmask": _bf16(_pm(np.asarray(p["mask_proj_w"], np.float32).T)),
        "bmask": _pm(np.asarray(p["mask_proj_b"], np.float32)[:, None]),
        "gs": _pm(np.ascontiguousarray(gate_s.T)),
        "gt": _pm(np.ascontiguousarray(gate_t.T)),
    }

    in_maps = []
    for ci in range(NC):
        dsl = slice(ci * DSH, (ci + 1) * DSH)
        m = dict(base)
        for mk_, mp in (("s", sem_mamba), ("t", tex_mamba)):
            wi = np.asarray(mp["in_proj_w"], np.float32)
            m[f"w_in_{mk_}"] = _bf16(_pm(np.ascontiguousarray(
                np.concatenate([wi[dsl], wi[DI + ci * DSH: DI + (ci + 1) * DSH]],
                               0).T)))
            wo = np.asarray(mp["out_proj_w"], np.float32)
            m[f"wout_{mk_}"] = _bf16(_pm(np.ascontiguousarray(wo[:, dsl].T)))
            for br, s in (("f", ""), ("b", "_b")):
                cname = mk_ + br
                xw = np.asarray(mp["xproj_w" + s], np.float32)[:, dsl]  # 96x256
                xw_pad = np.zeros((P, DSH), np.float32)
                xw_pad[:96] = xw
                m[f"xpw_{cname}"] = _bf16(_pm(np.ascontiguousarray(xw_pad.T)))
                dw = np.asarray(mp["dt_w" + s], np.float32)[dsl]  # 256 x 64
                dw_pad = np.zeros((P, DSH), np.float32)
                dw_pad[:DTRANK] = dw.T
                m[f"dtw_{cname}"] = _bf16(dw_pad[:, None, :])
                cw = np.asarray(mp["conv_w" + s], np.float32)[dsl]  # 256x4
                m[f"convw_{cname}"] = _pm(cw).astype(np.float32)
                m[f"convb_{cname}"] = _pm(
                    np.asarray(mp["conv_b" + s], np.float32)[dsl, None])
                m[f"dtb_{cname}"] = _pm(
                    np.asarray(mp["dt_bias" + s], np.float32)[dsl, None])
                m[f"A_{cname}"] = _pm(-np.exp(
                    np.asarray(mp["A_log" + s], np.float32)[dsl]))
                m[f"D_{cname}"] = _pm(
                    np.asarray(mp["D" + s], np.float32)[dsl, None])
        in_maps.append(m)

    key = "nc"
    if key not in _CACHE:
        _CACHE[key] = _build_bass(float(np.asarray(params["alpha_x"])),
                                  float(np.asarray(params["alpha_z"])))
    nc = _CACHE[key]

    from concourse.bass_utils import run_bass_kernel_spmd
    import os as _os
    res = run_bass_kernel_spmd(nc, in_maps, core_ids=list(range(NC)),
                               trace=bool(_os.environ.get("BASS_TRACE")))
    if res.exec_time_ns is not None:
        print(f"HW exec time: {res.exec_time_ns} ns")
    else:
        try:  # no NTFF hook under this axon client: report cost-model estimate
            from concourse.timeline_sim import TimelineSim
            est = TimelineSim(nc, trace=False).simulate()
            print(f"HW exec time: {est:.0f} ns (TimelineSim estimate)")
        except Exception:
            pass
    r0 = res.results[0]
    hs_out = _unpm(np.asarray(r0["out_sem"], np.float32)).T.reshape(B, L, DIM)
    ht_out = _unpm(np.asarray(r0["out_tex"], np.float32)).T.reshape(B, L, DIM)
    return hs_out, ht_out


# revision 28
# speedup vs baseline: 1.0339x; 1.0032x over previous
"""DualStreamStegoDiSBlock on 8 TRN2 NeuronCores (Bass/Tile).

Sharding: tensor-parallel over d_inner (DI=2048 -> 256 ch/core) for both
mambas; x_proj/out_proj produce partials that are AllReduced. Everything
else (pointwise matmuls, final gating) is computed redundantly on every
core; host reads core 0's output.

All DRAM matrices [R, C] are stored interleaved as [128, R//128, C] with
row r at (p=r%128, rr=r//128)  (matmul_tile_kernel's native layout).
"""

import sys

sys.path.insert(0, "/opt/trn_rl_repo")

import numpy as np

DIM = 1024
SECRET = 512
B = 4
L = 1024
DI = 2 * DIM
DSTATE = 16
DTRANK = DIM // 16
K = 4
EPS = 1e-5
NC = 8
DSH = DI // NC  # 256 d_inner channels per core
T = B * L  # 4096 tokens
P = 128


def _pm(a):
    """[R, C] -> [128, R//128, C] row-interleaved (r -> (r%128, r//128))."""
    r, c = a.shape
    assert r % P == 0, r
    return np.ascontiguousarray(a.reshape(r // P, P, c).transpose(1, 0, 2))


def _unpm(a):
    """inverse of _pm"""
    p, rr, c = a.shape
    return np.ascontiguousarray(a.transpose(1, 0, 2).reshape(rr * p, c))


def _bf16(a):
    import ml_dtypes

    return np.asarray(a, dtype=ml_dtypes.bfloat16)


def _silu_np(x):
    return x / (1.0 + np.exp(-x))


def _build_bass(alpha_x: float, alpha_z: float):
    import concourse.bass as bass
    import concourse.mybir as mybir
    import concourse.tile as tile
    from concourse import bacc
    from concourse.alu_op_type import AluOpType
    from concourse.kernels.tile_matmul import matmul_tile_kernel
    from concourse.masks import make_identity

    f32 = mybir.dt.float32
    bf16 = mybir.dt.bfloat16
    AF = mybir.ActivationFunctionType
    groups = [list(range(NC))]

    nc = bacc.Bacc(None, target_bir_lowering=False)

    with tile.TileContext(nc) as tc:
        dram = tc.alloc_tile_pool(name="dram", bufs=1, space="DRAM")

        def din(name, shape, dt=f32):
            return dram.tile(shape, dt, kind="ExternalInput", name=name,
                             uniquify=False)

        # ---- inputs (per-core content differs only for weight slices) ----
        smod = din("smod", [P, 8, T], bf16)      # sem_mod^T
        tmod = din("tmod", [P, 8, T], bf16)      # tex_mod^T
        sec = din("sec", [P, 4, T], bf16)        # secret^T
        hsem = din("hsem", [P, 8, T], bf16)      # h_sem^T
        htex = din("htex", [P, 8, T], bf16)      # h_tex^T
        w_in = {m: din(f"w_in_{m}", [P, 8, 2 * DSH], bf16) for m in "st"}
        cfgs = ["sf", "sb", "tf", "tb"]
        xpw = {c: din(f"xpw_{c}", [P, 2, P], bf16) for c in cfgs}
        dtw = {c: din(f"dtw_{c}", [P, 1, DSH], bf16) for c in cfgs}
        convw = {c: din(f"convw_{c}", [P, 2, K]) for c in cfgs}
        convb = {c: din(f"convb_{c}", [P, 2, 1]) for c in cfgs}
        dtb = {c: din(f"dtb_{c}", [P, 2, 1]) for c in cfgs}
        A_in = {c: din(f"A_{c}", [P, 2, DSTATE]) for c in cfgs}
        D_in = {c: din(f"D_{c}", [P, 2, 1]) for c in cfgs}
        wout = {m: din(f"wout_{m}", [P, 2, DIM], bf16) for m in "st"}
        wsx = din("wsx", [P, 4, DIM], bf16)
        bsx = din("bsx", [P, 8, 1])
        wsz = din("wsz", [P, 4, DIM], bf16)
        bsz = din("bsz", [P, 8, 1])
        wtz = din("wtz", [P, 8, DIM], bf16)
        btz = din("btz", [P, 8, 1])
        wmask = din("wmask", [P, 8, DIM], bf16)
        bmask = din("bmask", [P, 8, 1])
        gs = din("gs", [P, 8, B])
        gt = din("gt", [P, 8, B])

        out_sem = dram.tile([P, 8, T], f32, kind="ExternalOutput",
                            name="out_sem", uniquify=False)
        out_tex = dram.tile([P, 8, T], f32, kind="ExternalOutput",
                            name="out_tex", uniquify=False)

        # ---- DRAM intermediates ----
        xT = dram.tile([P, 8, T], bf16, name="xT")
        xz = {"s": dram.tile([P, 4, T], f32, name="xz_s"),
              "t": dram.tile([P, 4, T], f32, name="xz_t")}
        u_d = {c: dram.tile([P, 2, T], f32, name=f"u_{c}") for c in cfgs}
        dblps = {c: dram.tile([P, 1, T], f32, name=f"dblp_{c}")
                 for c in cfgs}
        dbl_ars = {c: dram.tile([P, 1, T], f32, name=f"dblar_{c}",
                                addr_space="Shared") for c in cfgs}
        delta = {c: dram.tile([P, 2, T], f32, name=f"delta_{c}") for c in cfgs}
        y_d = {c: dram.tile([P, 2, T], f32, name=f"y_{c}") for c in cfgs}
        ygate = {m: dram.tile([P, 2, T], bf16, name=f"ygate_{m}") for m in "st"}
        Oalls = {m: dram.tile([P, 8, T], bf16, name=f"Oall_{m}")
                 for m in "st"}
        Oars = {m: dram.tile([P, 8, T], bf16, name=f"Oar_{m}",
                             addr_space="Shared") for m in "st"}
        maskin = dram.tile([P, 8, T], bf16, name="maskin")
        maskT = dram.tile([P, 8, T], bf16, name="maskT")
        z1 = dram.tile([P, 8, T], f32, name="z1")
        z2 = dram.tile([P, 8, T], f32, name="z2")

        const = tc.alloc_tile_pool(name="const", bufs=1)

        _cn = [0]

        def _load_const(ap, shape, dt=f32):
            _cn[0] += 1
            t_ = const.tile(shape, dt, tag=f"c{_cn[0]}")
            nc.sync.dma_start(out=t_[:], in_=ap)
            return t_

        ones1 = const.tile([P, P], bf16)
        nc.any.memset(ones1[:], 1.0)
        onec = const.tile([P, 1], f32, tag="onec")
        nc.any.memset(onec[:], 1.0)
        ident = const.tile([P, P], bf16, tag="ident")
        make_identity(nc, ident)

        def post_act(bias_sb, func, scale_after=None):
            """post_mxn hook: sbuf[:, j, :] = func(sbuf[:, j, :] + bias[:, col])
            (bias_sb is an SBUF const tile [P, 8, 1])."""
            def fn(nc_, sbuf, md, _):
                msub = sbuf.shape[1] if len(sbuf.shape) == 3 else 1
                for j in range(msub):
                    col = (md.m_tile_idx * md.m_tile) // P + j
                    sl = sbuf[:, j, :] if len(sbuf.shape) == 3 else sbuf[:]
                    if func == "softplus":  # ln(1 + exp(x + b))
                        nc_.scalar.activation(sl, sl, AF.Exp,
                                              bias=bias_sb[:, col, :])
                        nc_.scalar.activation(sl, sl, AF.Ln, bias=onec[:])
                    else:
                        nc_.scalar.activation(sl, sl, func,
                                              bias=bias_sb[:, col, :])
                    if scale_after is not None:
                        nc_.scalar.mul(sl, sl, scale_after)
            return fn

        bsx_sb = _load_const(bsx[:], [P, 8, 1])
        bsz_sb = _load_const(bsz[:], [P, 8, 1])
        btz_sb = _load_const(btz[:], [P, 8, 1])
        bmask_sb = _load_const(bmask[:], [P, 8, 1])
        dtb_sb = {c: _load_const(dtb[c][:], [P, 2, 1]) for c in cfgs}
        convw_sb = {c: _load_const(convw[c][:], [P, 2, K]) for c in cfgs}
        convb_sb = {c: _load_const(convb[c][:], [P, 2, 1]) for c in cfgs}
        A_sb = {c: _load_const(A_in[c][:], [P, 2, DSTATE]) for c in cfgs}
        D_sb = {c: _load_const(D_in[c][:], [P, 2, 1]) for c in cfgs}
        gs_sb = _load_const(gs[:], [P, 8, B])
        gt_sb = _load_const(gt[:], [P, 8, B])

        # ---- phase 1: x = tmod + ax*tanh(Wsx @ sec + bsx) ----
        matmul_tile_kernel(tc, wsx[:], sec[:], xT[:], accumulate_ap=tmod[:],
                           post_mxn_tile_fn=post_act(bsx_sb, AF.Tanh, alpha_x))
        # in_proj (sem from smod, tex from x)
        matmul_tile_kernel(tc, w_in["s"][:], smod[:], xz["s"][:])
        matmul_tile_kernel(tc, w_in["t"][:], xT[:], xz["t"][:])
        matmul_tile_kernel(tc, wtz[:], tmod[:], z1[:],
                           post_mxn_tile_fn=post_act(btz_sb, AF.Identity))
        matmul_tile_kernel(tc, wsz[:], sec[:], z2[:],
                           post_mxn_tile_fn=post_act(bsz_sb, AF.Tanh, alpha_z))

        # ---- phase 2: causal depthwise conv + silu (bwd cfgs reversed) ----
        cpool = tc.alloc_tile_pool(name="conv", bufs=3)
        for c in cfgs:
            src = xz[c[0] == "t" and "t" or "s"]
            rev = c[1] == "b"
            for kk in range(2):
                for b in range(B):
                    sl = slice(b * L, (b + 1) * L)
                    X = cpool.tile([P, L + K - 1], f32, tag="X")
                    nc.any.memset(X[:, 0:K - 1], 0.0)
                    src_ap = src[:, kk, sl]
                    if rev:
                        src_ap = src_ap[:, ::-1]
                    nc.sync.dma_start(out=X[:, K - 1:], in_=src_ap)
                    acc = cpool.tile([P, L], f32, tag="acc")
                    nc.vector.tensor_scalar_mul(
                        acc[:], X[:, 0:L], convw_sb[c][:, kk, 0:1])
                    for k in range(1, K):
                        nc.vector.scalar_tensor_tensor(
                            acc[:], X[:, k:k + L], convw_sb[c][:, kk, k:k + 1],
                            acc[:], AluOpType.mult, AluOpType.add)
                    us = cpool.tile([P, L], f32, tag="us")
                    nc.scalar.activation(us[:], acc[:], AF.Silu,
                                         bias=convb_sb[c][:, kk, :])
                    nc.sync.dma_start(out=u_d[c][:, kk, sl], in_=us[:])

        # ---- phase 3: x_proj partials then AllReduce ----
        for i, c in enumerate(cfgs):
            matmul_tile_kernel(tc, xpw[c][:], u_d[c][:], dblps[c][:],
                               matmul_dtype=bf16)
            nc.gpsimd.collective_compute(
                "AllReduce", AluOpType.add, replica_groups=groups,
                ins=[dblps[c][:]], outs=[dbl_ars[c][:]])

        # ---- phase 4: delta = softplus(dtw @ dt + dtb) ----
        for i, c in enumerate(cfgs):
            matmul_tile_kernel(tc, dtw[c][:], dbl_ars[c][:],
                               delta[c][:], matmul_dtype=bf16,
                               post_mxn_tile_fn=post_act(dtb_sb[c],
                                                         "softplus"))

        # ---- phase 5: selective scan ----
        cpool.release()
        spool = tc.alloc_tile_pool(name="scan", bufs=3)
        sres = tc.alloc_tile_pool(name="scan_res", bufs=2)
        pp = tc.alloc_tile_pool(name="scan_psum", bufs=2, space="PSUM")
        ppy = tc.alloc_tile_pool(name="scan_psum_y", bufs=1, space="PSUM")
        for i, c in enumerate(cfgs):
            for b in range(B):
                sl = slice(b * L, (b + 1) * L)
                Bc = spool.tile([1, 2 * DSTATE, L], bf16, tag="Bc", bufs=1)
                nc.gpsimd.dma_start(out=Bc[:],
                                    in_=dbl_ars[c][DTRANK:DTRANK + 2 * DSTATE,
                                                   0, sl])
                dl, du, ya = [], [], []
                for kk in range(2):
                    d_ = sres.tile([P, L], f32, tag=f"d{kk}")
                    u_ = sres.tile([P, L], f32, tag=f"u{kk}")
                    nc.sync.dma_start(out=d_[:], in_=delta[c][:, kk, sl])
                    nc.sync.dma_start(out=u_[:], in_=u_d[c][:, kk, sl])
                    duk = sres.tile([P, L], f32, tag=f"du{kk}")
                    nc.vector.tensor_mul(duk[:], d_[:], u_[:])
                    y_ = ppy.tile([P, L], f32, tag=f"yp{kk}")
                    dl.append(d_); du.append(duk); ya.append(y_)
                for n in range(DSTATE):
                    pb = pp.tile([P, L], f32, tag="pb", bufs=1)
                    pc = pp.tile([P, L], f32, tag="pc", bufs=1)
                    for h in range(2):
                        hs = slice(h * 512, (h + 1) * 512)
                        nc.tensor.matmul(pb[:, hs], ones1[0:1, :],
                                         Bc[0:1, n, hs])
                        nc.tensor.matmul(pc[:, hs], ones1[0:1, :],
                                         Bc[0:1, DSTATE + n, hs])
                    for kk in range(2):
                        a_ = spool.tile([P, L], f32, tag="a")
                        nc.scalar.activation(a_[:], dl[kk][:], AF.Exp,
                                             scale=A_sb[c][:, kk, n:n + 1])
                        bt = spool.tile([P, L], f32, tag="bt")
                        nc.vector.tensor_mul(bt[:], du[kk][:], pb[:])
                        h_ = spool.tile([P, L], f32, tag="h")
                        nc.vector.tensor_tensor_scan(
                            h_[:], a_[:], bt[:], 0.0,
                            AluOpType.mult, AluOpType.add)
                        yc = spool.tile([P, L], bf16, tag="yc")
                        nc.vector.tensor_mul(yc[:], h_[:], pc[:])
                        first = n == 0
                        last = n == DSTATE - 1
                        for hh in range(2):
                            hs2 = slice(hh * 512, (hh + 1) * 512)
                            nc.tensor.matmul(ya[kk][:, hs2], ident[:],
                                             yc[:, hs2], start=first,
                                             stop=last)
                for kk in range(2):
                    yo = spool.tile([P, L], f32, tag="yo")
                    nc.scalar.copy(yo[:], ya[kk][:])
                    nc.sync.dma_start(out=y_d[c][:, kk, sl], in_=yo[:])

        # ---- phase 6: y_total = (yf + uf*Df + rev(yb + ub*Db)) * silu(z) ----
        ppy.release(); pp.release(); sres.release(); spool.release()
        gpool = tc.alloc_tile_pool(name="gate", bufs=3)
        for m in "st":
            cf, cb = m + "f", m + "b"
            for kk in range(2):
                for b in range(B):
                    sl = slice(b * L, (b + 1) * L)
                    yf = gpool.tile([P, L], f32, tag="yf")
                    uf = gpool.tile([P, L], f32, tag="uf")
                    yb = gpool.tile([P, L], f32, tag="yb")
                    ub = gpool.tile([P, L], f32, tag="ub")
                    nc.sync.dma_start(out=yf[:], in_=y_d[cf][:, kk, sl])
                    nc.sync.dma_start(out=uf[:], in_=u_d[cf][:, kk, sl])
                    nc.sync.dma_start(out=yb[:], in_=y_d[cb][:, kk, sl][:, ::-1])
                    nc.sync.dma_start(out=ub[:], in_=u_d[cb][:, kk, sl][:, ::-1])
                    nc.vector.scalar_tensor_tensor(
                        yf[:], uf[:], D_sb[cf][:, kk, :], yf[:],
                        AluOpType.mult, AluOpType.add)
                    nc.vector.scalar_tensor_tensor(
                        yb[:], ub[:], D_sb[cb][:, kk, :], yb[:],
                        AluOpType.mult, AluOpType.add)
                    nc.vector.tensor_add(yf[:], yf[:], yb[:])
                    zz = gpool.tile([P, L], f32, tag="zz")
                    nc.sync.dma_start(out=zz[:],
                                      in_=xz[m][:, 2 + kk, sl])
                    nc.scalar.activation(zz[:], zz[:], AF.Silu)
                    yg = gpool.tile([P, L], bf16, tag="yg")
                    nc.vector.tensor_mul(yg[:], yf[:], zz[:])
                    nc.sync.dma_start(out=ygate[m][:, kk, sl], in_=yg[:])

        # ---- phase 7: out_proj partials + AllReduce ----
        matmul_tile_kernel(tc, wout["s"][:], ygate["s"][:], Oalls["s"][:])
        nc.gpsimd.collective_compute(
            "AllReduce", AluOpType.add, replica_groups=groups,
            ins=[Oalls["s"][:]], outs=[Oars["s"][:]])
        matmul_tile_kernel(tc, wout["t"][:], ygate["t"][:], Oalls["t"][:])
        nc.gpsimd.collective_compute(
            "AllReduce", AluOpType.add, replica_groups=groups,
            ins=[Oalls["t"][:]], outs=[Oars["t"][:]])

        # ---- phase 8: h_sem_out (= mask input), mask, z, final gating ----
        gpool.release()
        fpool = tc.alloc_tile_pool(name="fin", bufs=3)
        for rr in range(8):
            for b in range(B):
                sl = slice(b * L, (b + 1) * L)
                o_ = fpool.tile([P, L], bf16, tag="o")
                hs = fpool.tile([P, L], bf16, tag="hs")
                nc.sync.dma_start(out=o_[:], in_=Oars["s"][:, rr, sl])
                nc.sync.dma_start(out=hs[:], in_=hsem[:, rr, sl])
                ho = fpool.tile([P, L], f32, tag="ho")
                nc.vector.scalar_tensor_tensor(
                    ho[:], o_[:], gs_sb[:, rr, b:b + 1], hs[:],
                    AluOpType.mult, AluOpType.add)
                nc.sync.dma_start(out=out_sem[:, rr, sl], in_=ho[:])
                hob = fpool.tile([P, L], bf16, tag="hob")
                nc.vector.tensor_copy(hob[:], ho[:])
                nc.sync.dma_start(out=maskin[:, rr, sl], in_=hob[:])

        matmul_tile_kernel(tc, wmask[:], maskin[:], maskT[:],
                           post_mxn_tile_fn=post_act(bmask_sb, AF.Sigmoid))

        for rr in range(8):
            for b in range(B):
                sl = slice(b * L, (b + 1) * L)
                za = fpool.tile([P, L], f32, tag="za")
                zb = fpool.tile([P, L], f32, tag="zb")
                nc.sync.dma_start(out=za[:], in_=z1[:, rr, sl])
                nc.sync.dma_start(out=zb[:], in_=z2[:, rr, sl])
                nc.vector.tensor_add(za[:], za[:], zb[:])
                nc.scalar.activation(za[:], za[:], AF.Silu)
                ot = fpool.tile([P, L], bf16, tag="ot")
                nc.sync.dma_start(out=ot[:], in_=Oars["t"][:, rr, sl])
                nc.vector.tensor_mul(ot[:], ot[:], za[:])
                mk = fpool.tile([P, L], bf16, tag="mk")
                nc.sync.dma_start(out=mk[:], in_=maskT[:, rr, sl])
                nc.vector.tensor_mul(ot[:], ot[:], mk[:])
                ht = fpool.tile([P, L], bf16, tag="ht")
                nc.sync.dma_start(out=ht[:], in_=htex[:, rr, sl])
                hto = fpool.tile([P, L], f32, tag="hto")
                nc.vector.scalar_tensor_tensor(
                    hto[:], ot[:], gt_sb[:, rr, b:b + 1], ht[:],
                    AluOpType.mult, AluOpType.add)
                nc.sync.dma_start(out=out_tex[:, rr, sl], in_=hto[:])

        fpool.release()
        const.release()

    nc.compile()
    return nc


_CACHE = {}


def kernel(h_sem, h_tex, c_global, secret_seq, params, sem_mamba, tex_mamba):
    p = params
    # ---- host: adaLN + LN + modulation (tiny fraction of FLOPs) ----
    c = _silu_np(np.asarray(c_global, np.float32))
    mod_s = c @ np.asarray(p["adaLN_sem_w"], np.float32).T + np.asarray(
        p["adaLN_sem_b"], np.float32)
    mod_t = c @ np.asarray(p["adaLN_tex_w"], np.float32).T + np.asarray(
        p["adaLN_tex_b"], np.float32)
    shift_s, scale_s, gate_s = np.split(mod_s, 3, -1)
    shift_t, scale_t, gate_t = np.split(mod_t, 3, -1)

    def ln_mod(h, w, b, scale, shift):
        h = np.asarray(h, np.float32)
        mu = h.mean(-1, keepdims=True)
        var = h.var(-1, keepdims=True)
        xh = (h - mu) / np.sqrt(var + EPS) * np.asarray(w, np.float32) + \
            np.asarray(b, np.float32)
        return xh * (1.0 + scale[:, None]) + shift[:, None]

    sem_mod = ln_mod(h_sem, p["norm_sem_w"], p["norm_sem_b"], scale_s, shift_s)
    tex_mod = ln_mod(h_tex, p["norm_tex_w"], p["norm_tex_b"], scale_t, shift_t)

    def tokmat(x):  # (B,L,D) -> D-major [P, D//P, T]
        d = x.shape[-1]
        return _pm(np.asarray(x, np.float32).reshape(T, d).T)

    smod_T = _bf16(tokmat(sem_mod))
    tmod_T = _bf16(tokmat(tex_mod))
    sec_T = _bf16(tokmat(secret_seq))
    hsem_T = _bf16(tokmat(h_sem))
    htex_T = _bf16(tokmat(h_tex))

    base = {
        "smod": smod_T, "tmod": tmod_T, "sec": sec_T,
        "hsem": hsem_T, "htex": htex_T,
        "wsx": _bf16(_pm(np.asarray(p["secret_to_x_w"], np.float32).T)),
        "bsx": _pm(np.asarray(p["secret_to_x_b"], np.float32)[:, None]),
        "wsz": _bf16(_pm(np.asarray(p["secret_to_z_w"], np.float32).T)),
        "bsz": _pm(np.asarray(p["secret_to_z_b"], np.float32)[:, None]),
        "wtz": _bf16(_pm(np.asarray(p["tex_to_z_w"], np.float32).T)),
        "btz": _pm(np.asarray(p["tex_to_z_b"], np.float32)[:, None]),
        "wmask": _bf16(_pm(np.asarray(p["mask_proj_w"], np.float32).T)),
        "bmask": _pm(np.asarray(p["mask_proj_b"], np.float32)[:, None]),
        "gs": _pm(np.ascontiguousarray(gate_s.T)),
        "gt": _pm(np.ascontiguousarray(gate_t.T)),
    }

    in_maps = []
    for ci in range(NC):
        dsl = slice(ci * DSH, (ci + 1) * DSH)
        m = dict(base)
        for mk_, mp in (("s", sem_mamba), ("t", tex_mamba)):
            wi = np.asarray(mp["in_proj_w"], np.float32)
            m[f"w_in_{mk_}"] = _bf16(_pm(np.ascontiguousarray(
                np.concatenate([wi[dsl], wi[DI + ci * DSH: DI + (ci + 1) * DSH]],
                               0).T)))
            wo = np.asarray(mp["out_proj_w"], np.float32)
            m[f"wout_{mk_}"] = _bf16(_pm(np.ascontiguousarray(wo[:, dsl].T)))
            for br, s in (("f", ""), ("b", "_b")):
                cname = mk_ + br
                xw = np.asarray(mp["xproj_w" + s], np.float32)[:, dsl]  # 96x256
                xw_pad = np.zeros((P, DSH), np.float32)
                xw_pad[:96] = xw
                m[f"xpw_{cname}"] = _bf16(_pm(np.ascontiguousarray(xw_pad.T)))
                dw = np.asarray(mp["dt_w" + s], np.float32)[dsl]  # 256 x 64
                dw_pad = np.zeros((P, DSH), np.float32)
                dw_pad[:DTRANK] = dw.T
                m[f"dtw_{cname}"] = _bf16(dw_pad[:, None, :])
                cw = np.asarray(mp["conv_w" + s], np.float32)[dsl]  # 256x4
                m[f"convw_{cname}"] = _pm(cw).astype(np.float32)
                m[f"convb_{cname}"] = _pm(
                    np.asarray(mp["conv_b" + s], np.float32)[dsl, None])
                m[f"dtb_{cname}"] = _pm(
                    np.asarray(mp["dt_bias" + s], np.float32)[dsl, None])
                m[f"A_{cname}"] = _pm(-np.exp(
                    np.asarray(mp["A_log" + s], np.float32)[dsl]))
                m[f"D_{cname}"] = _pm(
                    np.asarray(mp["D" + s], np.float32)[dsl, None])
        in_maps.append(m)

    key = "nc"
    if key not in _CACHE:
        _CACHE[key] = _build_bass(float(np.asarray(params["alpha_x"])),
                                  float(np.asarray(params["alpha_z"])))
    nc = _CACHE[key]

    from concourse.bass_utils import run_bass_kernel_spmd
    import os as _os
    res = run_bass_kernel_spmd(nc, in_maps, core_ids=list(range(NC)),
                               trace=bool(_os.environ.get("BASS_TRACE")))
    if res.exec_time_ns is not None:
        print(f"HW exec time: {res.exec_time_ns} ns")
    else:
        try:  # no NTFF hook under this axon client: report cost-model estimate
            from concourse.timeline_sim import TimelineSim
            est = TimelineSim(nc, trace=False).simulate()
            print(f"HW exec time: {est:.0f} ns (TimelineSim estimate)")
        except Exception:
            pass
    r0 = res.results[0]
    hs_out = _unpm(np.asarray(r0["out_sem"], np.float32)).T.reshape(B, L, DIM)
    ht_out = _unpm(np.asarray(r0["out_tex"], np.float32)).T.reshape(B, L, DIM)
    return hs_out, ht_out
